# revision 1
# baseline (speedup 1.0000x reference)
"""Bass/Trainium2 kernel for the GCL loss function (nn_GCL_46076409151702).

Math (reference):
    g_s = segment_sum(z_s, batch_s, 512)            s in {1,2}
    zn_s, gn_s = l2norm rows
    pos11 = <zn1[i], gn1[b1[i]]>, cross12 = <zn1[i], gn2[b1[i]]>
    pos22 = <zn2[i], gn2[b2[i]]>, cross21 = <zn2[i], gn1[b2[i]]>
    d_s = softplus(-cross) - softplus(-pos)
    out  = sqrt(sum d1^2) + sqrt(sum d2^2)

Strategy: data-parallel over nodes on 8 cores; AllReduce the per-graph sums.
Key data property: batch ids are *sorted*, and every graph has >=128 nodes,
so each 128-node tile touches at most 2 consecutive graphs {A_t, A_t+1}.
All index-dependent structures (per-tile 2-column masks, scatter matrix,
gather indices) are built host-side from batch and passed as input tensors,
so the compiled graph is input-independent.

Per-core device pipeline:
  P1  per 128-node tile: seg-matmul (lhsT=z_bf16, rhs=mask2 -> [d,2]
      contribs, accumulated region-wise into one PSUM bank per side),
      ACT square+accum (row norms)
  P2  scatter contribs -> g[512,128] via one-hot matmuls; AllReduce g1|g2;
      zT loaded via hardware DMA-transpose (overlaps the collective)
  P3  normalize g rows; P4 transpose to gT layout; P5 gather per-tile
      candidate columns (gpsimd indirect_copy)
  P6  per tile: matvec (lhsT=zT bf16, rhs=4 candidate cols) -> 4 dots/node,
      region-wise into 2 PSUM banks
  P7  select by mask, scale by row rsqrt, softplus, accumulate d^2; output
      per-core partial [2] -> host: sqrt(sum) + sqrt(sum)
"""

import numpy as np
import ml_dtypes

import concourse.bass as bass
import concourse.bacc as bacc
import concourse.mybir as mybir
import concourse.tile as tile
from concourse.bass_utils import run_bass_kernel_spmd
from concourse.masks import make_identity

F32 = mybir.dt.float32
BF16 = mybir.dt.bfloat16
U16 = mybir.dt.uint16
AL = mybir.AluOpType
AF = mybir.ActivationFunctionType

NCORES = 8
G = 512          # num graphs
D = 128          # feature dim
P = 128          # partitions
NT = 98          # tiles per core
R = NT * P       # rows per core = 12544
NPAD = NCORES * R  # 100352
CK = 2           # candidate graphs per tile (sorted batch, counts >= 128)
GW = G + 4       # per-side column width in gT table (pad col for A+1==512)
NIDX = NT * 2 * CK * 2  # gather cols: NT tiles x 2 sides x (CK pos + CK cross)
EPS = 1e-12
CH = 49          # tiles per z DMA group (1.57 MB per DMA)
NGRP = NT // CH  # 2
HT = 49          # tiles per P6 psum accumulator bank (49*8 f32 = 1568B)


def build_nc(finalize=True, stage=99):
    # stage (debug bisect): 0=inputs only, 1=P1, 2=+collective, 3=+normalize,
    # 4=+gather, 99=full.
    # Bacc (not plain Bass): its compile pipeline legalizes sync waits for
    # TRN2's one-wait-per-instruction constraint and allocates registers.
    nc = bacc.Bacc(None, target_bir_lowering=False, debug=False)
    z1 = nc.dram_tensor("z1", [R, D], BF16, kind="ExternalInput")
    z2 = nc.dram_tensor("z2", [R, D], BF16, kind="ExternalInput")
    # interleaved per-tile masks: mab[p, CK*t+j] = (batch[t*128+p] == A_t + j)
    mab1 = nc.dram_tensor("mab1", [P, NT * CK], BF16, kind="ExternalInput")
    mab2 = nc.dram_tensor("mab2", [P, NT * CK], BF16, kind="ExternalInput")
    # scatter one-hot: sel[c, g] = 1 iff A_{c//CK} + c%CK == g (c < NT*CK)
    sel1 = nc.dram_tensor("sel1", [2 * P, G], F32, kind="ExternalInput")
    sel2 = nc.dram_tensor("sel2", [2 * P, G], F32, kind="ExternalInput")
    # indirect_copy wrapped indices into the [P, 2*GW] gT table
    gidx = nc.dram_tensor("gidx", [P, NIDX // 16], U16, kind="ExternalInput")
    out_part = nc.dram_tensor("out_part", [2, 1], F32, kind="ExternalOutput")

    zs = [z1, z2]
    mabs = [mab1, mab2]
    sels = [sel1, sel2]

    def _body(tc):
        with (
            tc.tile_pool(name="const", bufs=1) as constp,
            tc.tile_pool(name="stash", bufs=1) as stashp,
            tc.tile_pool(name="zin", bufs=2) as zinp,
            tc.tile_pool(name="scr", bufs=4) as scrp,
            tc.tile_pool(name="small", bufs=4) as smallp,
            tc.tile_pool(name="pstag", bufs=1, space="PSUM") as pstagp,
            tc.tile_pool(name="pcand", bufs=1, space="PSUM") as pcandp,
            tc.tile_pool(name="pzt", bufs=3, space="PSUM") as pztp,
            tc.tile_pool(name="dram", bufs=1, space="DRAM") as dramp,
        ):
            # ---- constants / inputs to SBUF ----
            ident = constp.tile([P, P], F32)
            make_identity(nc, ident[:])
            ones_col = constp.tile([P, 1], F32)
            nc.vector.memset(ones_col[:], 1.0)
            ones_bf = constp.tile([P, 1], BF16)
            nc.vector.memset(ones_bf[:], 1.0)

            mab_sb = []
            sel_sb = []
            for s in range(2):
                m = constp.tile([P, NT * CK], BF16, name=f"mab_sb{s}")
                nc.sync.dma_start(out=m[:], in_=mabs[s][:])
                mab_sb.append(m)
                s0 = constp.tile([P, G], F32, name=f"sel_sb{s}a")
                s1 = constp.tile([P, G], F32, name=f"sel_sb{s}b")
                nc.sync.dma_start(out=s0[:], in_=sels[s][0:P, :])
                nc.sync.dma_start(out=s1[:], in_=sels[s][P : 2 * P, :])
                sel_sb.append((s0, s1))
            gidx_sb = constp.tile([P, NIDX // 16], U16)
            nc.sync.dma_start(out=gidx_sb[:], in_=gidx[:])

            if stage <= 0:
                osb0 = smallp.tile([2, 1], F32, tag="osb")
                nc.vector.tensor_copy(out=osb0[:], in_=ones_col[0:2, 0:1])
                nc.sync.dma_start(out=out_part[:], in_=osb0[:])
                return

            # ---- persistent stashes ----
            zst = [stashp.tile([P, NT * P], BF16, name=f"zst{s}") for s in range(2)]
            stagT = [stashp.tile([P, NT * CK], F32, name=f"stagT{s}") for s in range(2)]
            ssq = [stashp.tile([P, NT], F32, name=f"ssq{s}") for s in range(2)]
            # cand8: per tile 8 cols = [s0:posA,posB,crossA,crossB | s1:...]
            cand8 = stashp.tile([P, NT * 8], F32, name="cand8")
            sqscr = stashp.tile([P, NT * P], BF16, name="sqscr")

            pstag = [
                pstagp.tile([P, NT * CK], F32, name=f"pstag{s}") for s in range(2)
            ]

            # ---- P1: stream z tiles; seg matmuls into region-wise PSUM ----
            for g in range(NGRP):
                for s in range(2):
                    zg = zinp.tile([P, CH * P], BF16, tag=f"zg{s}")
                    zr = zs[s].rearrange("(a k p) d -> a p k d", a=NGRP, k=CH, p=P)
                    nc.sync.dma_start(
                        out=zg[:].rearrange("p (k d) -> p k d", k=CH), in_=zr[g]
                    )
                    for k in range(CH):
                        t = g * CH + k
                        zt = zg[:, k * P : (k + 1) * P]
                        nc.tensor.matmul(
                            out=pstag[s][:, CK * t : CK * (t + 1)],
                            lhsT=zt,
                            rhs=mab_sb[s][:, CK * t : CK * (t + 1)],
                            start=True,
                            stop=True,
                        )
                        nc.scalar.activation(
                            out=sqscr[:, t * P : (t + 1) * P],
                            in_=zt,
                            func=AF.Square,
                            accum_out=ssq[s][:, t : t + 1],
                        )
            for s in range(2):
                nc.vector.tensor_copy(out=stagT[s][:], in_=pstag[s][:])

            if stage <= 1:
                osb1 = smallp.tile([2, 1], F32, tag="osb")
                nc.vector.tensor_copy(out=osb1[:], in_=stagT[0][0:2, 0:1])
                nc.sync.dma_start(out=out_part[:], in_=osb1[:])
                return

            # ---- P2: scatter contribs to g and AllReduce ----
            gloc = []
            for s in range(2):
                # staging [c, d] = transpose of stagT
                stg_a = smallp.tile([P, P], F32, tag="stg", bufs=2)
                stg_b = smallp.tile([P, P], F32, tag="stg", bufs=2)
                pta = pztp.tile([P, P], F32, tag="pzt")
                nc.tensor.transpose(out=pta[:], in_=stagT[s][:, 0:P], identity=ident[:])
                nc.vector.tensor_copy(out=stg_a[:], in_=pta[:])
                rem = NT * CK - P  # 68
                ptb = pztp.tile([P, P], F32, tag="pzt")
                nc.tensor.transpose(
                    out=ptb[:rem, :], in_=stagT[s][:, P : NT * CK], identity=ident[:]
                )
                nc.vector.tensor_copy(out=stg_b[:rem, :], in_=ptb[:rem, :])

                gl = stashp.tile([P, 4 * P], F32, name=f"gloc{s}")
                for gc in range(4):
                    pg = pztp.tile([P, P], F32, tag="pzt")
                    nc.tensor.matmul(
                        out=pg[:],
                        lhsT=sel_sb[s][0][:, gc * P : (gc + 1) * P],
                        rhs=stg_a[:],
                        start=True,
                        stop=False,
                    )
                    nc.tensor.matmul(
                        out=pg[:],
                        lhsT=sel_sb[s][1][:rem, gc * P : (gc + 1) * P],
                        rhs=stg_b[:rem, :],
                        start=False,
                        stop=True,
                    )
                    nc.vector.tensor_copy(out=gl[:, gc * P : (gc + 1) * P], in_=pg[:])
                gloc.append(gl)

            drin = dramp.tile([2, 4, P, P], F32)
            drout = dramp.tile([2, 4, P, P], F32, addr_space="Shared")
            for s in range(2):
                nc.sync.dma_start(
                    out=drin[s].rearrange("c p d -> p c d"),
                    in_=gloc[s][:].rearrange("p (c d) -> p c d", c=4),
                )
            nc.gpsimd.collective_compute(
                "AllReduce",
                AL.add,
                replica_groups=[list(range(NCORES))],
                ins=[drin.opt()],
                outs=[drout.opt()],
            )

            # zT stash via hardware DMA transpose (overlaps the collective)
            for s in range(2):
                nc.sync.dma_start_transpose(out=zst[s][:], in_=zs[s][:])

            if stage <= 2:
                g0 = smallp.tile([P, 4 * P], F32, tag="gsum", bufs=2)
                nc.sync.dma_start(
                    out=g0[:].rearrange("p (c d) -> p c d", c=4),
                    in_=drout[0].rearrange("c p d -> p c d"),
                )
                osb2 = smallp.tile([2, 1], F32, tag="osb")
                nc.vector.tensor_copy(out=osb2[:], in_=g0[0:2, 0:1])
                nc.sync.dma_start(out=out_part[:], in_=osb2[:])
                return

            # ---- P3/P4: normalize g rows, build gT table ----
            gtab = stashp.tile([P, 2 * GW], F32, name="gtab")
            nc.vector.memset(gtab[:], 0.0)
            for s in range(2):
                gsum = smallp.tile([P, 4 * P], F32, tag="gsum", bufs=2)
                nc.sync.dma_start(
                    out=gsum[:].rearrange("p (c d) -> p c d", c=4),
                    in_=drout[s].rearrange("c p d -> p c d"),
                )
                for gc in range(4):
                    chunk = gsum[:, gc * P : (gc + 1) * P]
                    sqg = scrp.tile([P, P], F32, tag="sq")
                    ss = smallp.tile([P, 1], F32, tag="nrm")
                    nc.scalar.activation(
                        out=sqg[:], in_=chunk, func=AF.Square, accum_out=ss[:]
                    )
                    nrm = smallp.tile([P, 1], F32, tag="nrm")
                    nc.scalar.activation(out=nrm[:], in_=ss[:], func=AF.Sqrt)
                    nc.vector.tensor_scalar(
                        out=nrm[:], in0=nrm[:], scalar1=EPS, scalar2=None, op0=AL.max
                    )
                    inv = smallp.tile([P, 1], F32, tag="nrm")
                    nc.vector.reciprocal(out=inv[:], in_=nrm[:])
                    nc.vector.tensor_scalar(
                        out=chunk,
                        in0=chunk,
                        scalar1=inv[:, 0:1],
                        scalar2=None,
                        op0=AL.mult,
                    )
                    pt = pztp.tile([P, P], F32, tag="pzt")
                    nc.tensor.transpose(out=pt[:], in_=chunk, identity=ident[:])
                    nc.vector.tensor_copy(
                        out=gtab[:, s * GW + gc * P : s * GW + (gc + 1) * P],
                        in_=pt[:],
                    )

            if stage <= 3:
                osb3 = smallp.tile([2, 1], F32, tag="osb")
                nc.vector.tensor_copy(out=osb3[:], in_=gtab[0:2, 0:1])
                nc.sync.dma_start(out=out_part[:], in_=osb3[:])
                return

            # ---- P5: gather candidate columns ----
            gsel_f = stashp.tile([P, NIDX], F32, name="gsel_f")
            nc.gpsimd.indirect_copy(gsel_f[:], gtab[:], gidx_sb[:], True)
            gsel = stashp.tile([P, NIDX], BF16, name="gsel")
            nc.vector.tensor_copy(out=gsel[:], in_=gsel_f[:])

            if stage <= 4:
                osb4 = smallp.tile([2, 1], F32, tag="osb")
                nc.vector.tensor_copy(out=osb4[:], in_=gsel[0:2, 0:1])
                nc.sync.dma_start(out=out_part[:], in_=osb4[:])
                return

            # ---- P6: per-tile matvecs, region-wise into 2 PSUM banks ----
            pcand = [
                pcandp.tile([P, HT * 8], F32, name=f"pcand{h}") for h in range(2)
            ]
            for t in range(NT):
                h, o = t // HT, (t % HT) * 8
                for s in range(2):
                    nc.tensor.matmul(
                        out=pcand[h][:, o + 4 * s : o + 4 * s + 4],
                        lhsT=zst[s][:, t * P : (t + 1) * P],
                        rhs=gsel[:, (t * 2 + s) * 4 : (t * 2 + s) * 4 + 4],
                        start=True,
                        stop=True,
                    )
            for h in range(2):
                nc.vector.tensor_copy(
                    out=cand8[:, h * HT * 8 : (h + 1) * HT * 8], in_=pcand[h][:]
                )

            # ---- P7: select, scale, softplus, reduce ----
            mabf = [stashp.tile([P, NT * CK], F32, name=f"mabf{s}") for s in range(2)]
            for s in range(2):
                nc.vector.tensor_copy(out=mabf[s][:], in_=mab_sb[s][:])
            d2col = smallp.tile([P, 2], F32, tag="d2col")
            for s in range(2):
                rn = smallp.tile([P, NT], F32, tag=f"fin{s}")
                nc.scalar.activation(out=rn[:], in_=ssq[s][:], func=AF.Sqrt)
                nc.vector.tensor_scalar(
                    out=rn[:], in0=rn[:], scalar1=EPS, scalar2=None, op0=AL.max
                )
                inv = smallp.tile([P, NT], F32, tag=f"fin{s}")
                nc.vector.reciprocal(out=inv[:], in_=rn[:])

                cv = cand8[:].rearrange("p (t w) -> p w t", w=8)
                mv = mabf[s][:].rearrange("p (t j) -> p j t", j=CK)
                quant = []
                for base in (4 * s, 4 * s + 2):  # pos cols, cross cols
                    ta = smallp.tile([P, NT], F32, tag=f"fin{s}")
                    tb = smallp.tile([P, NT], F32, tag=f"fin{s}")
                    nc.vector.tensor_tensor(
                        out=ta[:], in0=mv[:, 0, :], in1=cv[:, base, :], op=AL.mult
                    )
                    nc.vector.tensor_tensor(
                        out=tb[:], in0=mv[:, 1, :], in1=cv[:, base + 1, :], op=AL.mult
                    )
                    nc.vector.tensor_tensor(out=ta[:], in0=ta[:], in1=tb[:], op=AL.add)
                    # scale by per-node reciprocal norm, then
                    # softplus(-x) = -ln(sigmoid(x)); d uses the ln form
                    nc.vector.tensor_tensor(out=ta[:], in0=ta[:], in1=inv[:], op=AL.mult)
                    sg = smallp.tile([P, NT], F32, tag=f"fin{s}")
                    nc.scalar.activation(out=sg[:], in_=ta[:], func=AF.Sigmoid)
                    ll = smallp.tile([P, NT], F32, tag=f"fin{s}")
                    nc.scalar.activation(out=ll[:], in_=sg[:], func=AF.Ln)
                    quant.append(ll)
                # d = sp(-cross) - sp(-pos) = ln(sig(pos)) - ln(sig(cross))
                dd = smallp.tile([P, NT], F32, tag=f"fin{s}")
                nc.vector.tensor_tensor(
                    out=dd[:], in0=quant[0][:], in1=quant[1][:], op=AL.subtract
                )
                dsq = scrp.tile([P, NT], F32, tag="dsq")
                nc.scalar.activation(
                    out=dsq[:], in_=dd[:], func=AF.Square,
                    accum_out=d2col[:, s : s + 1],
                )

            pfin = pztp.tile([2, 1], F32, tag="pzt")
            nc.tensor.matmul(
                out=pfin[:], lhsT=d2col[:], rhs=ones_col[:], start=True, stop=True
            )
            osb = smallp.tile([2, 1], F32, tag="osb")
            nc.vector.tensor_copy(out=osb[:], in_=pfin[:])
            nc.sync.dma_start(out=out_part[:], in_=osb[:])

    with tile.TileContext(nc) as tc:
        _body(tc)
    if finalize:
        nc.finalize()
    return nc


def prep_inputs(z1, z2, batch_1, batch_2):
    """Pad/shard host-side and build all index-derived input tensors."""
    z1 = np.asarray(z1, dtype=np.float32)
    z2 = np.asarray(z2, dtype=np.float32)
    b1 = np.asarray(batch_1).astype(np.int64)
    b2 = np.asarray(batch_2).astype(np.int64)
    n = z1.shape[0]
    assert n <= NPAD, n

    def pad_z(z):
        out = np.zeros((NPAD, D), dtype=ml_dtypes.bfloat16)
        out[:n] = z.astype(ml_dtypes.bfloat16)
        return out

    def pad_b(b):
        out = np.full((NPAD,), G - 1, dtype=np.int64)
        out[:n] = b
        return out

    z1p, z2p, b1p, b2p = pad_z(z1), pad_z(z2), pad_b(b1), pad_b(b2)

    in_maps = []
    for c in range(NCORES):
        lo, hi = c * R, (c + 1) * R
        m = {"z1": z1p[lo:hi], "z2": z2p[lo:hi]}
        idx_cols = np.zeros((NT, 2, 2 * CK), dtype=np.int64)
        for s, b in ((0, b1p[lo:hi]), (1, b2p[lo:hi])):
            bt = b.reshape(NT, P)
            A = bt[:, 0]  # [NT]
            span = bt[:, -1] - bt[:, 0]
            assert span.max() <= CK - 1, (
                f"tile graph span {span.max()} exceeds CK-1; regenerate with larger CK"
            )
            # masks [P, NT*CK] interleaved
            mab = np.zeros((P, NT * CK), dtype=ml_dtypes.bfloat16)
            for j in range(CK):
                mab[:, j::CK] = (bt == (A + j)[:, None]).T.astype(ml_dtypes.bfloat16)
            m[f"mab{s + 1}"] = mab
            # scatter one-hot [2P, G]
            sel = np.zeros((2 * P, G), dtype=np.float32)
            crow = np.arange(NT * CK)
            gid = np.repeat(A, CK) + np.tile(np.arange(CK), NT)
            ok = gid < G
            sel[crow[ok], gid[ok]] = 1.0
            m[f"sel{s + 1}"] = sel
            # gather columns: for side s, order = [own@A.., other@A..]
            own, other = s, 1 - s
            idx_cols[:, s, :CK] = own * GW + (A[:, None] + np.arange(CK))
            idx_cols[:, s, CK:] = other * GW + (A[:, None] + np.arange(CK))
        flat = idx_cols.reshape(-1)  # [NIDX] in (t, s, q) order
        assert flat.max() < 2 * GW
        # indirect_copy wrapped layout: output col j reads the index at
        # partition (group*16 + j%16), free col j//16; same for every group.
        wrapped = np.zeros((P, NIDX // 16), dtype=np.uint16)
        for cg in range(8):
            wrapped[cg * 16 : (cg + 1) * 16, :] = flat.reshape(-1, 16).T
        m["gidx"] = wrapped
        in_maps.append(m)
    return in_maps


_NC_CACHE = {}


def _get_nc():
    if "nc" not in _NC_CACHE:
        _NC_CACHE["nc"] = build_nc()
    return _NC_CACHE["nc"]


def kernel(z1, z2, batch_1, batch_2):
    nc = _get_nc()
    in_maps = prep_inputs(z1, z2, batch_1, batch_2)
    res = run_bass_kernel_spmd(nc, in_maps, list(range(NCORES)))
    parts = np.stack([r["out_part"].reshape(2) for r in res.results])  # [8, 2]
    tot = parts.sum(axis=0)
    return np.float32(np.sqrt(tot[0]) + np.sqrt(tot[1]))



# revision 5
# speedup vs baseline: 2.1863x; 2.1863x over previous
"""Bass/Trainium2 kernel for the GCL loss function (nn_GCL_46076409151702).

Math (reference):
    g_s = segment_sum(z_s, batch_s, 512)            s in {1,2}
    zn_s, gn_s = l2norm rows
    pos11 = <zn1[i], gn1[b1[i]]>, cross12 = <zn1[i], gn2[b1[i]]>
    pos22 = <zn2[i], gn2[b2[i]]>, cross21 = <zn2[i], gn1[b2[i]]>
    d_s = softplus(-cross) - softplus(-pos)
    out  = sqrt(sum d1^2) + sqrt(sum d2^2)

Sharding: batch ids are SORTED, so splitting the 512 graphs into 8
contiguous blocks of 64 gives each core a contiguous row range whose
segment sums are fully local -- NO collective at all.  Each core handles
graphs [64c, 64c+64); its row shard (per side) is exactly the rows whose
batch falls in that window, zero-padded to a fixed NT*128.

Per-core pipeline (all phases overlap the chunked z DMAs):
  P1  per 128-node tile: seg-matmul (lhsT=z_bf16 tile, rhs=2-col mask)
      accumulated region-wise into one PSUM tile per side
  S   row sums of squares: Scalar squares z chunks, DVE tensor_reduce
      per-tile row sums (no per-tile READ_ACCUMULATOR)
  P2  transpose stag, scatter into local g[64,128] via one-hot matmuls
  P3  normalize g rows; transpose into gT table [128, 2*68]
  P5  gather per-tile candidate columns (gpsimd indirect_copy)
  P6  per tile: matvec (lhsT=zT tile from host-transposed input,
      rhs=4 candidate cols) into 2 PSUM banks
  P7  select by mask, scale by row rsqrt, softplus, accumulate d^2;
      per-core partial [2] -> host: sqrt(sum) + sqrt(sum)
"""

import numpy as np
import ml_dtypes

import concourse.bass as bass
import concourse.bacc as bacc
import concourse.mybir as mybir
import concourse.tile as tile
from concourse.bass_utils import run_bass_kernel_spmd
from concourse.masks import make_identity

F32 = mybir.dt.float32
BF16 = mybir.dt.bfloat16
U16 = mybir.dt.uint16
AL = mybir.AluOpType
AF = mybir.ActivationFunctionType
AX = mybir.AxisListType

NCORES = 8
G = 512          # num graphs
GPC = G // NCORES  # graphs per core = 64
D = 128          # feature dim
P = 128          # partitions
NT = 103         # tiles per core (12500 expected rows + slack)
R = NT * P       # rows per core = 13184
CK = 2           # candidate graphs per tile (sorted batch)
GW = GPC + 4     # per-side column width in gT table = 68
NIDX = NT * 2 * CK * 2   # 824 gather cols
NIDXP = 832      # padded to multiple of 16
EPS = 1e-12
HT = 52          # tiles per P6 psum bank
CHT = [13] * 7 + [12]    # z DMA chunk sizes in tiles


def build_nc(finalize=True):
    nc = bacc.Bacc(None, target_bir_lowering=False, debug=False)
    z1 = nc.dram_tensor("z1", [R, D], BF16, kind="ExternalInput")
    z2 = nc.dram_tensor("z2", [R, D], BF16, kind="ExternalInput")
    zt1 = nc.dram_tensor("zt1", [P, R], BF16, kind="ExternalInput")
    zt2 = nc.dram_tensor("zt2", [P, R], BF16, kind="ExternalInput")
    # interleaved per-tile masks: mab[p, CK*t+j] = (batch[t*128+p] == A_t + j)
    mab1 = nc.dram_tensor("mab1", [P, NT * CK], BF16, kind="ExternalInput")
    mab2 = nc.dram_tensor("mab2", [P, NT * CK], BF16, kind="ExternalInput")
    # scatter one-hot: sel[c, w] = 1 iff a_{c//CK} + c%CK == w (local window)
    sel1 = nc.dram_tensor("sel1", [2 * P, GPC], F32, kind="ExternalInput")
    sel2 = nc.dram_tensor("sel2", [2 * P, GPC], F32, kind="ExternalInput")
    # indirect_copy wrapped indices into the [P, 2*GW] gT table
    gidx = nc.dram_tensor("gidx", [P, NIDXP // 16], U16, kind="ExternalInput")
    out_part = nc.dram_tensor("out_part", [2, 1], F32, kind="ExternalOutput")

    zs = [z1, z2]
    zts = [zt1, zt2]
    mabs = [mab1, mab2]
    sels = [sel1, sel2]

    def _body(tc):
        with (
            tc.tile_pool(name="const", bufs=1) as constp,
            tc.tile_pool(name="stash", bufs=1) as stashp,
            tc.tile_pool(name="zin", bufs=3) as zinp,
            tc.tile_pool(name="sq", bufs=2) as sqp,
            tc.tile_pool(name="small", bufs=4) as smallp,
            tc.tile_pool(name="pstag", bufs=1, space="PSUM") as pstagp,
            tc.tile_pool(name="pcand", bufs=1, space="PSUM") as pcandp,
            tc.tile_pool(name="pg", bufs=1, space="PSUM") as pgp,
            tc.tile_pool(name="pzt", bufs=3, space="PSUM") as pztp,
        ):
            # ---- constants / small inputs to SBUF (sync queue) ----
            ident = constp.tile([P, P], F32)
            make_identity(nc, ident[:])
            ones_col = constp.tile([P, 1], F32)
            nc.vector.memset(ones_col[:], 1.0)

            mab_sb = []
            sel_sb = []
            for s in range(2):
                m = constp.tile([P, NT * CK], BF16, name=f"mab_sb{s}")
                nc.sync.dma_start(out=m[:], in_=mabs[s][:])
                mab_sb.append(m)
                s0 = constp.tile([P, GPC], F32, name=f"sel_sb{s}a")
                s1 = constp.tile([P, GPC], F32, name=f"sel_sb{s}b")
                nc.sync.dma_start(out=s0[:], in_=sels[s][0:P, :])
                nc.sync.dma_start(out=s1[:], in_=sels[s][P : 2 * P, :])
                sel_sb.append((s0, s1))
            gidx_sb = constp.tile([P, NIDXP // 16], U16)
            nc.sync.dma_start(out=gidx_sb[:], in_=gidx[:])

            # ---- persistent stashes ----
            zst = [stashp.tile([P, R], BF16, name=f"zst{s}") for s in range(2)]
            ssq = stashp.tile([P, 2 * NT], F32, name="ssq")
            gtab = stashp.tile([P, 2 * GW], F32, name="gtab")
            nc.gpsimd.memset(gtab[:], 0.0)
            cand8 = stashp.tile([P, NT * 8], F32, name="cand8")

            # ---- DMA dispatch: z natural chunks (sync), zT chunks (vector) ----
            znat_bufs = []   # (s, chunk) -> sbuf tile [P, w*P]
            t0 = 0
            zview = [z.rearrange("(t p) d -> p t d", p=P) for z in zs]
            for gi, w in enumerate(CHT):
                row = []
                for s in range(2):
                    zg = zinp.tile([P, 13 * P], BF16, tag=f"zn{s}")
                    nc.sync.dma_start(
                        out=zg[:, : w * P].rearrange("p (k d) -> p k d", k=w),
                        in_=zview[s][:, t0 : t0 + w, :],
                    )
                    row.append(zg)
                znat_bufs.append((t0, w, row))
                t0 += w
            t0 = 0
            for gi, w in enumerate(CHT):
                for s in range(2):
                    nc.gpsimd.dma_start(
                        out=zst[s][:, t0 * P : (t0 + w) * P],
                        in_=zts[s][:, t0 * P : (t0 + w) * P],
                    )
                t0 += w

            # ---- P1: seg matmuls into region-wise PSUM (tensor queue) ----
            pstag = [
                pstagp.tile([P, NT * CK], F32, name=f"pstag{s}") for s in range(2)
            ]
            for t0, w, row in znat_bufs:
                for s in range(2):
                    for k in range(w):
                        t = t0 + k
                        nc.tensor.matmul(
                            out=pstag[s][:, CK * t : CK * (t + 1)],
                            lhsT=row[s][:, k * P : (k + 1) * P],
                            rhs=mab_sb[s][:, CK * t : CK * (t + 1)],
                            start=True,
                            stop=True,
                        )

            # ---- S: squares (scalar) + per-tile row sums (vector reduce) ----
            for t0, w, row in znat_bufs:
                for s in range(2):
                    sq = sqp.tile([P, 13 * P], BF16, tag="sq")
                    nc.scalar.activation(
                        out=sq[:, : w * P], in_=row[s][:, : w * P], func=AF.Square
                    )
                    nc.vector.tensor_reduce(
                        out=ssq[:, s * NT + t0 : s * NT + t0 + w],
                        in_=sq[:, : w * P].rearrange("p (k d) -> p k d", k=w),
                        axis=AX.X,
                        op=AL.add,
                    )

            # ---- P2: transpose stag, scatter into local g ----
            pg = pgp.tile([GPC, 2 * P], F32, name="pg")
            for s in range(2):
                stag_sb = smallp.tile([P, NT * CK], F32, tag="stag", bufs=2)
                nc.scalar.copy(out=stag_sb[:], in_=pstag[s][:])
                rem = NT * CK - P  # 78
                stg_a = smallp.tile([P, P], F32, tag="stg", bufs=2)
                stg_b = smallp.tile([P, P], F32, tag="stg", bufs=2)
                pta = pztp.tile([P, P], F32, tag="pzt")
                nc.tensor.transpose(out=pta[:], in_=stag_sb[:, 0:P], identity=ident[:])
                nc.scalar.copy(out=stg_a[:], in_=pta[:])
                ptb = pztp.tile([P, P], F32, tag="pzt")
                nc.tensor.transpose(
                    out=ptb[:rem, :], in_=stag_sb[:, P : NT * CK], identity=ident[:]
                )
                nc.scalar.copy(out=stg_b[:rem, :], in_=ptb[:rem, :])
                nc.tensor.matmul(
                    out=pg[:, s * P : (s + 1) * P],
                    lhsT=sel_sb[s][0][:],
                    rhs=stg_a[:],
                    start=True,
                    stop=False,
                )
                nc.tensor.matmul(
                    out=pg[:, s * P : (s + 1) * P],
                    lhsT=sel_sb[s][1][:rem, :],
                    rhs=stg_b[:rem, :],
                    start=False,
                    stop=True,
                )

            # ---- P3: normalize g rows, build gT table ----
            gf = smallp.tile([GPC, 2 * P], F32, tag="gf")
            nc.scalar.copy(out=gf[:], in_=pg[:])
            for s in range(2):
                chunk = gf[:, s * P : (s + 1) * P]
                gsq = smallp.tile([GPC, P], F32, tag="gsq", bufs=2)
                gss = smallp.tile([GPC, 1], F32, tag="gnrm", bufs=8)
                nc.scalar.activation(
                    out=gsq[:], in_=chunk, func=AF.Square, accum_out=gss[:]
                )
                gnr = smallp.tile([GPC, 1], F32, tag="gnrm", bufs=8)
                nc.scalar.sqrt(out=gnr[:], in_=gss[:])
                nc.vector.tensor_scalar(
                    out=gnr[:], in0=gnr[:], scalar1=EPS, scalar2=None, op0=AL.max
                )
                ginv = smallp.tile([GPC, 1], F32, tag="gnrm", bufs=8)
                nc.vector.reciprocal(out=ginv[:], in_=gnr[:])
                nc.scalar.mul(out=chunk, in_=chunk, mul=ginv[:, 0:1])
                pt = pztp.tile([P, GPC], F32, tag="pzt")
                nc.tensor.transpose(
                    out=pt[:, :], in_=chunk, identity=ident[0:GPC, 0:GPC]
                )
                nc.scalar.copy(
                    out=gtab[:, s * GW : s * GW + GPC], in_=pt[:, :]
                )

            # ---- P5: gather candidate columns ----
            gsel_f = stashp.tile([P, NIDXP], F32, name="gsel_f")
            nc.gpsimd.indirect_copy(gsel_f[:], gtab[:], gidx_sb[:], True)
            gselb = stashp.tile([P, NIDXP], BF16, name="gselb")
            nc.vector.tensor_copy(out=gselb[:], in_=gsel_f[:])

            # ---- P6: per-tile matvecs, region-wise into 2 PSUM banks ----
            pcand = [
                pcandp.tile([P, HT * 8], F32, name=f"pcand{h}") for h in range(2)
            ]
            for t in range(NT):
                h, o = t // HT, (t % HT) * 8
                for s in range(2):
                    nc.tensor.matmul(
                        out=pcand[h][:, o + 4 * s : o + 4 * s + 4],
                        lhsT=zst[s][:, t * P : (t + 1) * P],
                        rhs=gselb[:, (t * 2 + s) * 4 : (t * 2 + s) * 4 + 4],
                        start=True,
                        stop=True,
                    )
            nc.vector.tensor_copy(
                out=cand8[:, : HT * 8], in_=pcand[0][:]
            )
            nc.vector.tensor_copy(
                out=cand8[:, HT * 8 : NT * 8], in_=pcand[1][:, : (NT - HT) * 8]
            )

            # ---- P7: select, scale, softplus, reduce ----
            rn = smallp.tile([P, 2 * NT], F32, tag="rn", bufs=2)
            nc.scalar.sqrt(out=rn[:], in_=ssq[:])
            nc.vector.tensor_scalar(
                out=rn[:], in0=rn[:], scalar1=EPS, scalar2=None, op0=AL.max
            )
            inv = smallp.tile([P, 2 * NT], F32, tag="rn", bufs=2)
            nc.vector.reciprocal(out=inv[:], in_=rn[:])
            mabf = [
                smallp.tile([P, NT * CK], F32, tag=f"mabf{s}", bufs=1, name=f"mabf{s}")
                for s in range(2)
            ]
            for s in range(2):
                nc.vector.tensor_copy(out=mabf[s][:], in_=mab_sb[s][:])
            d2col = smallp.tile([P, 2], F32, tag="d2col")
            for s in range(2):
                cv = cand8[:].rearrange("p (t w) -> p w t", w=8)
                mv = mabf[s][:].rearrange("p (t j) -> p j t", j=CK)
                iv = inv[:, s * NT : (s + 1) * NT]
                quant = []
                for base in (4 * s, 4 * s + 2):  # pos cols, cross cols
                    ta = smallp.tile([P, NT], F32, tag=f"fin{s}", bufs=8)
                    tb = smallp.tile([P, NT], F32, tag=f"fin{s}", bufs=8)
                    nc.vector.tensor_tensor(
                        out=ta[:], in0=mv[:, 0, :], in1=cv[:, base, :], op=AL.mult
                    )
                    nc.vector.tensor_tensor(
                        out=tb[:], in0=mv[:, 1, :], in1=cv[:, base + 1, :], op=AL.mult
                    )
                    nc.vector.tensor_tensor(out=ta[:], in0=ta[:], in1=tb[:], op=AL.add)
                    nc.vector.tensor_tensor(out=ta[:], in0=ta[:], in1=iv, op=AL.mult)
                    sg = smallp.tile([P, NT], F32, tag=f"fin{s}", bufs=8)
                    nc.scalar.activation(out=sg[:], in_=ta[:], func=AF.Sigmoid)
                    ll = smallp.tile([P, NT], F32, tag=f"fin{s}", bufs=8)
                    nc.scalar.activation(out=ll[:], in_=sg[:], func=AF.Ln)
                    quant.append(ll)
                # d = sp(-cross) - sp(-pos) = ln(sig(pos)) - ln(sig(cross))
                dd = smallp.tile([P, NT], F32, tag=f"fin{s}", bufs=8)
                nc.vector.tensor_tensor(
                    out=dd[:], in0=quant[0][:], in1=quant[1][:], op=AL.subtract
                )
                dsq = smallp.tile([P, NT], F32, tag="dsq", bufs=2)
                nc.scalar.activation(
                    out=dsq[:], in_=dd[:], func=AF.Square,
                    accum_out=d2col[:, s : s + 1],
                )

            pfin = pztp.tile([2, 1], F32, tag="pzt")
            nc.tensor.matmul(
                out=pfin[:], lhsT=d2col[:], rhs=ones_col[:], start=True, stop=True
            )
            osb = smallp.tile([2, 1], F32, tag="osb")
            nc.scalar.copy(out=osb[:], in_=pfin[:])
            nc.sync.dma_start(out=out_part[:], in_=osb[:])

    with tile.TileContext(nc) as tc:
        _body(tc)
    if finalize:
        nc.finalize()
    return nc


def prep_inputs(z1, z2, batch_1, batch_2):
    """Graph-aligned shards + all index-derived input tensors (host-side)."""
    z1 = np.asarray(z1, dtype=np.float32)
    z2 = np.asarray(z2, dtype=np.float32)
    b1 = np.asarray(batch_1).astype(np.int64)
    b2 = np.asarray(batch_2).astype(np.int64)

    in_maps = []
    for c in range(NCORES):
        glo, ghi = c * GPC, (c + 1) * GPC
        m = {}
        idx_cols = np.zeros((NT, 2, 2 * CK), dtype=np.int64)
        for s, (z, b) in enumerate(((z1, b1), (z2, b2))):
            lo, hi = np.searchsorted(b, [glo, ghi])
            cnt = hi - lo
            assert cnt <= R, f"core {c} side {s}: {cnt} rows > {R}"
            zp = np.zeros((R, D), dtype=ml_dtypes.bfloat16)
            zp[:cnt] = z[lo:hi].astype(ml_dtypes.bfloat16)
            m[f"z{s + 1}"] = zp
            m[f"zt{s + 1}"] = np.ascontiguousarray(zp.T)
            bt = np.full((R,), -1, dtype=np.int64)
            bt[:cnt] = b[lo:hi]
            btt = bt.reshape(NT, P)
            A = btt[:, 0].copy()
            A[A < 0] = glo
            vmax = btt.max(axis=1)
            assert (vmax - A <= CK - 1).all(), "tile spans >CK graphs"
            a = A - glo
            assert (a >= 0).all() and (a < GPC).all()
            mab = np.zeros((P, NT * CK), dtype=ml_dtypes.bfloat16)
            for j in range(CK):
                mab[:, j::CK] = (btt == (A + j)[:, None]).T.astype(
                    ml_dtypes.bfloat16
                )
            m[f"mab{s + 1}"] = mab
            sel = np.zeros((2 * P, GPC), dtype=np.float32)
            crow = np.arange(NT * CK)
            gid = np.repeat(a, CK) + np.tile(np.arange(CK), NT)
            ok = gid < GPC
            sel[crow[ok], gid[ok]] = 1.0
            m[f"sel{s + 1}"] = sel
            own, other = s, 1 - s
            idx_cols[:, s, :CK] = own * GW + (a[:, None] + np.arange(CK))
            idx_cols[:, s, CK:] = other * GW + (a[:, None] + np.arange(CK))
        flat = np.zeros((NIDXP,), dtype=np.int64)
        flat[:NIDX] = idx_cols.reshape(-1)
        assert flat.max() < 2 * GW
        wrapped = np.zeros((P, NIDXP // 16), dtype=np.uint16)
        for cg in range(8):
            wrapped[cg * 16 : (cg + 1) * 16, :] = flat.reshape(-1, 16).T
        m["gidx"] = wrapped
        in_maps.append(m)
    return in_maps


_NC_CACHE = {}


def _get_nc():
    if "nc" not in _NC_CACHE:
        _NC_CACHE["nc"] = build_nc()
    return _NC_CACHE["nc"]


def kernel(z1, z2, batch_1, batch_2):
    nc = _get_nc()
    in_maps = prep_inputs(z1, z2, batch_1, batch_2)
    res = run_bass_kernel_spmd(nc, in_maps, list(range(NCORES)))
    parts = np.stack([r["out_part"].reshape(2) for r in res.results])  # [8, 2]
    tot = parts.sum(axis=0)
    return np.float32(np.sqrt(tot[0]) + np.sqrt(tot[1]))


# revision 7
# speedup vs baseline: 3.5455x; 1.6217x over previous
"""Bass/Trainium2 kernel for the GCL loss function (nn_GCL_46076409151702).

Math (reference):
    g_s = segment_sum(z_s, batch_s, 512)            s in {1,2}
    zn_s, gn_s = l2norm rows
    pos11 = <zn1[i], gn1[b1[i]]>, cross12 = <zn1[i], gn2[b1[i]]>
    pos22 = <zn2[i], gn2[b2[i]]>, cross21 = <zn2[i], gn1[b2[i]]>
    d_s = softplus(-cross) - softplus(-pos)
    out  = sqrt(sum d1^2) + sqrt(sum d2^2)

Sharding: batch ids are SORTED, so splitting the 512 graphs into 8
contiguous blocks of 64 gives each core a contiguous row range whose
segment sums are fully local -- NO collective at all.  Each core handles
graphs [64c, 64c+64); its row shard (per side) is exactly the rows whose
batch falls in that window, zero-padded to a fixed NT*128 rows.

All z inputs arrive in two layouts (host-prepared, both contiguous per
partition): tile-interleaved natural zn[p, t*128+d] = z[t*128+p, d] for
the segment matmuls, and transposed zt[d, i] = z[i, d] for the dot
matvecs.  Everything overlaps the chunked z DMA stream:

  P1  per 128-node tile: seg-matmul (lhsT=z tile, rhs=2-col mask) into
      region-wise PSUM
  S   row sums of squares: Scalar squares z chunks, DVE tensor_reduce
      per-tile row sums
  P2  transpose stag, scatter into local g[64,128] via one-hot matmuls
  P3  normalize g rows (per-partition scale), cast to bf16
  P5  gather candidate columns via one-hot gather-matmuls (no gpsimd)
  P6  per tile: matvec (lhsT=zT tile, rhs=4 candidate cols) into 2 banks
  P7  select by mask, scale by row rsqrt, softplus, accumulate d^2;
      per-core partial [2] -> host: sqrt(sum) + sqrt(sum)
"""

import numpy as np
import ml_dtypes

import concourse.bass as bass
import concourse.bacc as bacc
import concourse.mybir as mybir
import concourse.tile as tile
from concourse.bass_utils import run_bass_kernel_spmd
from concourse.masks import make_identity

F32 = mybir.dt.float32
BF16 = mybir.dt.bfloat16
AL = mybir.AluOpType
AF = mybir.ActivationFunctionType
AX = mybir.AxisListType

NCORES = 8
G = 512          # num graphs
GPC = G // NCORES  # graphs per core = 64
D = 128          # feature dim
P = 128          # partitions
NT = 103         # tiles per core (12500 expected rows + slack)
R = NT * P       # rows per core = 13184
CK = 2           # candidate graphs per tile (sorted batch)
NIDX = NT * 2 * CK * 2   # 824 gather cols
NIDXP = 832      # padded
EPS = 1e-12
HT = 52          # tiles per P6 psum bank
CHT = [13] * 7 + [12]    # z DMA chunk sizes in tiles


def build_nc(finalize=True):
    nc = bacc.Bacc(None, target_bir_lowering=False, debug=False)
    # tile-interleaved natural layout: zn[p, t*128+d] = z[t*128+p, d]
    z1 = nc.dram_tensor("z1", [P, R], BF16, kind="ExternalInput")
    z2 = nc.dram_tensor("z2", [P, R], BF16, kind="ExternalInput")
    zt1 = nc.dram_tensor("zt1", [P, R], BF16, kind="ExternalInput")
    zt2 = nc.dram_tensor("zt2", [P, R], BF16, kind="ExternalInput")
    # interleaved per-tile masks: mab[p, CK*t+j] = (batch[t*128+p] == A_t + j)
    mab1 = nc.dram_tensor("mab1", [P, NT * CK], BF16, kind="ExternalInput")
    mab2 = nc.dram_tensor("mab2", [P, NT * CK], BF16, kind="ExternalInput")
    # scatter one-hot: sel[c, w] = 1 iff a_{c//CK} + c%CK == w (local window)
    sel1 = nc.dram_tensor("sel1", [2 * P, GPC], F32, kind="ExternalInput")
    sel2 = nc.dram_tensor("sel2", [2 * P, GPC], F32, kind="ExternalInput")
    # gather one-hot: e_s[w, j] = 1 iff candidate col j sources side s graph w
    e1 = nc.dram_tensor("e1", [GPC, NIDXP], BF16, kind="ExternalInput")
    e2 = nc.dram_tensor("e2", [GPC, NIDXP], BF16, kind="ExternalInput")
    out_part = nc.dram_tensor("out_part", [2, 1], F32, kind="ExternalOutput")

    zs = [z1, z2]
    zts = [zt1, zt2]
    mabs = [mab1, mab2]
    sels = [sel1, sel2]
    es = [e1, e2]

    def _body(tc):
        with (
            tc.tile_pool(name="const", bufs=1) as constp,
            tc.tile_pool(name="stash", bufs=1) as stashp,
            tc.tile_pool(name="zin", bufs=3) as zinp,
            tc.tile_pool(name="sq", bufs=2) as sqp,
            tc.tile_pool(name="small", bufs=4) as smallp,
            tc.tile_pool(name="pstag", bufs=1, space="PSUM") as pstagp,
            tc.tile_pool(name="pcand", bufs=1, space="PSUM") as pcandp,
            tc.tile_pool(name="pg", bufs=1, space="PSUM") as pgp,
            tc.tile_pool(name="pzt", bufs=3, space="PSUM") as pztp,
        ):
            # ---- constants / small inputs to SBUF (sync queue) ----
            ident = constp.tile([P, P], F32)
            make_identity(nc, ident[:])
            ones_col = constp.tile([P, 1], F32)
            nc.vector.memset(ones_col[:], 1.0)

            mab_sb = []
            sel_sb = []
            e_sb = []
            for s in range(2):
                m = constp.tile([P, NT * CK], BF16, name=f"mab_sb{s}")
                nc.sync.dma_start(out=m[:], in_=mabs[s][:])
                mab_sb.append(m)
                s0 = constp.tile([P, GPC], F32, name=f"sel_sb{s}a")
                s1 = constp.tile([P, GPC], F32, name=f"sel_sb{s}b")
                nc.sync.dma_start(out=s0[:], in_=sels[s][0:P, :])
                nc.sync.dma_start(out=s1[:], in_=sels[s][P : 2 * P, :])
                sel_sb.append((s0, s1))
                e = constp.tile([GPC, NIDXP], BF16, name=f"e_sb{s}")
                nc.sync.dma_start(out=e[:], in_=es[s][:])
                e_sb.append(e)

            # ---- persistent stashes ----
            zst = [stashp.tile([P, R], BF16, name=f"zst{s}") for s in range(2)]
            ssq = stashp.tile([P, 2 * NT], F32, name="ssq")
            cand8 = stashp.tile([P, NT * 8], F32, name="cand8")
            gselb = stashp.tile([P, NIDXP], BF16, name="gselb")

            # ---- DMA dispatch: z natural chunks first, then zT (all sync) ----
            znat_bufs = []   # (t0, w, [tile per side])
            t0 = 0
            for gi, w in enumerate(CHT):
                row = []
                for s in range(2):
                    zg = zinp.tile([P, 13 * P], BF16, tag=f"zn{s}")
                    nc.sync.dma_start(
                        out=zg[:, : w * P], in_=zs[s][:, t0 * P : (t0 + w) * P]
                    )
                    row.append(zg)
                znat_bufs.append((t0, w, row))
                t0 += w
            t0 = 0
            for gi, w in enumerate(CHT):
                for s in range(2):
                    nc.sync.dma_start(
                        out=zst[s][:, t0 * P : (t0 + w) * P],
                        in_=zts[s][:, t0 * P : (t0 + w) * P],
                    )
                t0 += w

            # ---- P1: seg matmuls into region-wise PSUM (tensor queue) ----
            pstag = [
                pstagp.tile([P, NT * CK], F32, name=f"pstag{s}") for s in range(2)
            ]
            for t0, w, row in znat_bufs:
                for s in range(2):
                    for k in range(w):
                        t = t0 + k
                        nc.tensor.matmul(
                            out=pstag[s][:, CK * t : CK * (t + 1)],
                            lhsT=row[s][:, k * P : (k + 1) * P],
                            rhs=mab_sb[s][:, CK * t : CK * (t + 1)],
                            start=True,
                            stop=True,
                        )

            # ---- S: squares (scalar) + per-tile row sums (vector reduce) ----
            for t0, w, row in znat_bufs:
                for s in range(2):
                    sq = sqp.tile([P, 13 * P], BF16, tag="sq")
                    nc.scalar.activation(
                        out=sq[:, : w * P], in_=row[s][:, : w * P], func=AF.Square
                    )
                    nc.vector.tensor_reduce(
                        out=ssq[:, s * NT + t0 : s * NT + t0 + w],
                        in_=sq[:, : w * P].rearrange("p (k d) -> p k d", k=w),
                        axis=AX.X,
                        op=AL.add,
                    )

            # ---- P2: transpose stag, scatter into local g ----
            pg = pgp.tile([GPC, 2 * P], F32, name="pg")
            for s in range(2):
                stag_sb = smallp.tile([P, NT * CK], F32, tag="stag", bufs=2)
                nc.scalar.copy(out=stag_sb[:], in_=pstag[s][:])
                rem = NT * CK - P  # 78
                stg_a = smallp.tile([P, P], F32, tag="stg", bufs=2)
                stg_b = smallp.tile([P, P], F32, tag="stg", bufs=2)
                pta = pztp.tile([P, P], F32, tag="pzt")
                nc.tensor.transpose(out=pta[:], in_=stag_sb[:, 0:P], identity=ident[:])
                nc.scalar.copy(out=stg_a[:], in_=pta[:])
                ptb = pztp.tile([P, P], F32, tag="pzt")
                nc.tensor.transpose(
                    out=ptb[:rem, :], in_=stag_sb[:, P : NT * CK], identity=ident[:]
                )
                nc.scalar.copy(out=stg_b[:rem, :], in_=ptb[:rem, :])
                nc.tensor.matmul(
                    out=pg[:, s * P : (s + 1) * P],
                    lhsT=sel_sb[s][0][:],
                    rhs=stg_a[:],
                    start=True,
                    stop=False,
                )
                nc.tensor.matmul(
                    out=pg[:, s * P : (s + 1) * P],
                    lhsT=sel_sb[s][1][:rem, :],
                    rhs=stg_b[:rem, :],
                    start=False,
                    stop=True,
                )

            # ---- P3: normalize g rows, cast to bf16 ----
            gf = smallp.tile([GPC, 2 * P], F32, tag="gf")
            nc.scalar.copy(out=gf[:], in_=pg[:])
            gnb = smallp.tile([GPC, 2 * P], BF16, tag="gnb")
            for s in range(2):
                chunk = gf[:, s * P : (s + 1) * P]
                gsq = smallp.tile([GPC, P], F32, tag="gsq", bufs=2)
                gss = smallp.tile([GPC, 1], F32, tag="gnrm", bufs=8)
                nc.scalar.activation(
                    out=gsq[:], in_=chunk, func=AF.Square, accum_out=gss[:]
                )
                gnr = smallp.tile([GPC, 1], F32, tag="gnrm", bufs=8)
                nc.scalar.sqrt(out=gnr[:], in_=gss[:])
                nc.vector.tensor_scalar(
                    out=gnr[:], in0=gnr[:], scalar1=EPS, scalar2=None, op0=AL.max
                )
                ginv = smallp.tile([GPC, 1], F32, tag="gnrm", bufs=8)
                nc.vector.reciprocal(out=ginv[:], in_=gnr[:])
                nc.scalar.mul(
                    out=gnb[:, s * P : (s + 1) * P], in_=chunk, mul=ginv[:, 0:1]
                )

            # ---- P5: gather candidate columns via one-hot matmuls ----
            H = NIDXP // 2  # 416
            for h in range(2):
                pgs = pztp.tile([P, H], F32, tag="pzt")
                for s in range(2):
                    nc.tensor.matmul(
                        out=pgs[:],
                        lhsT=gnb[:, s * P : (s + 1) * P],
                        rhs=e_sb[s][:, h * H : (h + 1) * H],
                        start=(s == 0),
                        stop=(s == 1),
                    )
                if h == 0:
                    nc.scalar.copy(out=gselb[:, h * H : (h + 1) * H], in_=pgs[:])
                else:
                    nc.vector.tensor_copy(
                        out=gselb[:, h * H : (h + 1) * H], in_=pgs[:]
                    )

            # ---- P6: per-tile matvecs, region-wise into 2 PSUM banks ----
            pcand = [
                pcandp.tile([P, HT * 8], F32, name=f"pcand{h}") for h in range(2)
            ]
            for t in range(NT):
                h, o = t // HT, (t % HT) * 8
                for s in range(2):
                    nc.tensor.matmul(
                        out=pcand[h][:, o + 4 * s : o + 4 * s + 4],
                        lhsT=zst[s][:, t * P : (t + 1) * P],
                        rhs=gselb[:, (t * 2 + s) * 4 : (t * 2 + s) * 4 + 4],
                        start=True,
                        stop=True,
                    )
            nc.vector.tensor_copy(out=cand8[:, : HT * 8], in_=pcand[0][:])
            nc.vector.tensor_copy(
                out=cand8[:, HT * 8 : NT * 8], in_=pcand[1][:, : (NT - HT) * 8]
            )

            # ---- P7: select, scale, softplus, reduce ----
            rn = smallp.tile([P, 2 * NT], F32, tag="rn", bufs=2)
            nc.scalar.sqrt(out=rn[:], in_=ssq[:])
            nc.vector.tensor_scalar(
                out=rn[:], in0=rn[:], scalar1=EPS, scalar2=None, op0=AL.max
            )
            inv = smallp.tile([P, 2 * NT], F32, tag="rn", bufs=2)
            nc.vector.reciprocal(out=inv[:], in_=rn[:])
            mabf = [
                smallp.tile([P, NT * CK], F32, tag=f"mabf{s}", bufs=1, name=f"mabf{s}")
                for s in range(2)
            ]
            for s in range(2):
                nc.vector.tensor_copy(out=mabf[s][:], in_=mab_sb[s][:])
            d2col = smallp.tile([P, 2], F32, tag="d2col")
            cv = cand8[:].rearrange("p (t w) -> p w t", w=8)
            # selected dots, scaled by per-node reciprocal norm
            tsel = []
            for s in range(2):
                mv = mabf[s][:].rearrange("p (t j) -> p j t", j=CK)
                iv = inv[:, s * NT : (s + 1) * NT]
                for base in (4 * s, 4 * s + 2):  # pos cols, cross cols
                    ta = smallp.tile([P, NT], F32, tag="fin", bufs=10)
                    tb = smallp.tile([P, NT], F32, tag="fin", bufs=10)
                    nc.vector.tensor_tensor(
                        out=ta[:], in0=mv[:, 0, :], in1=cv[:, base, :], op=AL.mult
                    )
                    nc.vector.tensor_tensor(
                        out=tb[:], in0=mv[:, 1, :], in1=cv[:, base + 1, :], op=AL.mult
                    )
                    nc.vector.tensor_tensor(out=ta[:], in0=ta[:], in1=tb[:], op=AL.add)
                    nc.vector.tensor_tensor(out=ta[:], in0=ta[:], in1=iv, op=AL.mult)
                    tsel.append(ta)
            # batched sigmoid then ln (fewer act-table reloads)
            sgs = []
            for k in range(4):
                sg = smallp.tile([P, NT], F32, tag="fin", bufs=10)
                nc.scalar.activation(out=sg[:], in_=tsel[k][:], func=AF.Sigmoid)
                sgs.append(sg)
            lls = []
            for k in range(4):
                ll = smallp.tile([P, NT], F32, tag="fin", bufs=10)
                nc.scalar.activation(out=ll[:], in_=sgs[k][:], func=AF.Ln)
                lls.append(ll)
            for s in range(2):
                # d = sp(-cross) - sp(-pos) = ln(sig(pos)) - ln(sig(cross))
                dd = smallp.tile([P, NT], F32, tag="fin", bufs=10)
                nc.vector.tensor_tensor(
                    out=dd[:], in0=lls[2 * s][:], in1=lls[2 * s + 1][:],
                    op=AL.subtract,
                )
                dsq = smallp.tile([P, NT], F32, tag="dsq", bufs=2)
                nc.scalar.activation(
                    out=dsq[:], in_=dd[:], func=AF.Square,
                    accum_out=d2col[:, s : s + 1],
                )

            pfin = pztp.tile([2, 1], F32, tag="pzt")
            nc.tensor.matmul(
                out=pfin[:], lhsT=d2col[:], rhs=ones_col[:], start=True, stop=True
            )
            osb = smallp.tile([2, 1], F32, tag="osb")
            nc.vector.tensor_copy(out=osb[:], in_=pfin[:])
            nc.sync.dma_start(out=out_part[:], in_=osb[:])

    with tile.TileContext(nc) as tc:
        _body(tc)
    if finalize:
        nc.finalize()
    return nc


def prep_inputs(z1, z2, batch_1, batch_2):
    """Graph-aligned shards + all index-derived input tensors (host-side)."""
    z1 = np.asarray(z1, dtype=np.float32)
    z2 = np.asarray(z2, dtype=np.float32)
    b1 = np.asarray(batch_1).astype(np.int64)
    b2 = np.asarray(batch_2).astype(np.int64)

    in_maps = []
    for c in range(NCORES):
        glo, ghi = c * GPC, (c + 1) * GPC
        m = {}
        idx_cols = np.zeros((NT, 2, 2 * CK), dtype=np.int64)
        for s, (z, b) in enumerate(((z1, b1), (z2, b2))):
            lo, hi = np.searchsorted(b, [glo, ghi])
            cnt = hi - lo
            assert cnt <= R, f"core {c} side {s}: {cnt} rows > {R}"
            zp = np.zeros((R, D), dtype=ml_dtypes.bfloat16)
            zp[:cnt] = z[lo:hi].astype(ml_dtypes.bfloat16)
            # tile-interleaved natural layout [128, R]
            m[f"z{s + 1}"] = np.ascontiguousarray(
                zp.reshape(NT, P, D).transpose(1, 0, 2).reshape(P, R)
            )
            m[f"zt{s + 1}"] = np.ascontiguousarray(zp.T)
            bt = np.full((R,), -1, dtype=np.int64)
            bt[:cnt] = b[lo:hi]
            btt = bt.reshape(NT, P)
            A = btt[:, 0].copy()
            A[A < 0] = glo
            vmax = btt.max(axis=1)
            assert (vmax - A <= CK - 1).all(), "tile spans >CK graphs"
            a = A - glo
            assert (a >= 0).all() and (a < GPC).all()
            mab = np.zeros((P, NT * CK), dtype=ml_dtypes.bfloat16)
            for j in range(CK):
                mab[:, j::CK] = (btt == (A + j)[:, None]).T.astype(
                    ml_dtypes.bfloat16
                )
            m[f"mab{s + 1}"] = mab
            sel = np.zeros((2 * P, GPC), dtype=np.float32)
            crow = np.arange(NT * CK)
            gid = np.repeat(a, CK) + np.tile(np.arange(CK), NT)
            ok = gid < GPC
            sel[crow[ok], gid[ok]] = 1.0
            m[f"sel{s + 1}"] = sel
            # candidate columns j = t*8 + s*4 + q: [own A, own A+1, other A, other A+1]
            idx_cols[:, s, :CK] = a[:, None] + np.arange(CK)          # from side s
            idx_cols[:, s, CK:] = a[:, None] + np.arange(CK)          # from side 1-s
        # gather one-hots: E_s[w, j] = 1 iff col j sources side s, graph w
        for s in range(2):
            E = np.zeros((GPC, NIDXP), dtype=ml_dtypes.bfloat16)
            for t in range(NT):
                for side in range(2):
                    for q in range(2 * CK):
                        src = side if q < CK else 1 - side
                        if src != s:
                            continue
                        w = idx_cols[t, side, q]
                        if w < GPC:
                            E[w, t * 8 + side * 4 + q] = 1.0
            m[f"e{s + 1}"] = E
        in_maps.append(m)
    return in_maps


_NC_CACHE = {}


def _get_nc():
    if "nc" not in _NC_CACHE:
        _NC_CACHE["nc"] = build_nc()
    return _NC_CACHE["nc"]


def kernel(z1, z2, batch_1, batch_2):
    nc = _get_nc()
    in_maps = prep_inputs(z1, z2, batch_1, batch_2)
    res = run_bass_kernel_spmd(nc, in_maps, list(range(NCORES)))
    parts = np.stack([r["out_part"].reshape(2) for r in res.results])  # [8, 2]
    tot = parts.sum(axis=0)
    return np.float32(np.sqrt(tot[0]) + np.sqrt(tot[1]))


# revision 10
# speedup vs baseline: 3.9650x; 1.1183x over previous
"""Bass/Trainium2 kernel for the GCL loss function (nn_GCL_46076409151702).

Math (reference):
    g_s = segment_sum(z_s, batch_s, 512)            s in {1,2}
    zn_s, gn_s = l2norm rows
    pos11 = <zn1[i], gn1[b1[i]]>, cross12 = <zn1[i], gn2[b1[i]]>
    pos22 = <zn2[i], gn2[b2[i]]>, cross21 = <zn2[i], gn1[b2[i]]>
    d_s = softplus(-cross) - softplus(-pos)
    out  = sqrt(sum d1^2) + sqrt(sum d2^2)

Sharding: batch ids are SORTED, so splitting the 512 graphs into 8
contiguous blocks of 64 gives each core a contiguous row range whose
segment sums are fully local -- NO collective at all.  Each core handles
graphs [64c, 64c+64); its row shard (per side) is exactly the rows whose
batch falls in that window, zero-padded to a fixed NT*128 rows.

z ships in fp8 e4m3 (error ~1e-3 on the final loss, gate is 2e-2) in two
host-prepared layouts, both contiguous per partition: tile-interleaved
natural zn[p, t*128+d] = z[t*128+p, d] for the segment matmuls, and
transposed zt[d, i] = z[i, d] for the dot matvecs.  Everything overlaps
the chunked DMA stream:

  P1  per 128-node tile: seg-matmul (lhsT=z tile, rhs=2-col fp8 mask)
  S   row norms: squares on Scalar (side 0) / DVE (side 1), row sums via
      per-tile ones-matmuls on Tensor (sq tile as weights)
  P2  transpose stag, scatter into local g[64,128] via one-hot matmuls
  P3  normalize g rows (per-partition scale), cast bf16
  P5  gather candidate columns via one-hot gather-matmuls, cast fp8
  P6  per tile: matvec (lhsT=zT fp8 tile, rhs=4 candidate cols)
  P7  select by mask (folded with 1/||z||), single batched Softplus,
      accumulate d^2; per-core partial [2] -> host: sqrt + sqrt
"""

import numpy as np
import ml_dtypes

import concourse.bass as bass
import concourse.bacc as bacc
import concourse.mybir as mybir
import concourse.tile as tile
from concourse.bass_utils import run_bass_kernel_spmd
from concourse.masks import make_identity

F32 = mybir.dt.float32
BF16 = mybir.dt.bfloat16
FP8 = mybir.dt.float8e4
AL = mybir.AluOpType
AF = mybir.ActivationFunctionType

NCORES = 8
G = 512          # num graphs
GPC = G // NCORES  # graphs per core = 64
D = 128          # feature dim
P = 128          # partitions
NT = 103         # tiles per core (12500 expected rows + slack)
R = NT * P       # rows per core = 13184
CK = 2           # candidate graphs per tile (sorted batch)
NIDX = NT * 2 * CK * 2   # 824 gather cols
NIDXP = 832      # padded
EPS = 1e-12
HT = 52          # tiles per P6 psum bank
CHN = [17, 17, 17, 17, 17, 18]   # z natural chunk sizes (tiles)
CHT = [34, 34, 35]               # zT chunk sizes (tiles)


def build_nc(finalize=True):
    nc = bacc.Bacc(None, target_bir_lowering=False, debug=False)
    # tile-interleaved natural layout: zn[p, t*128+d] = z[t*128+p, d]
    z1 = nc.dram_tensor("z1", [P, R], FP8, kind="ExternalInput")
    z2 = nc.dram_tensor("z2", [P, R], FP8, kind="ExternalInput")
    zt1 = nc.dram_tensor("zt1", [P, R], FP8, kind="ExternalInput")
    zt2 = nc.dram_tensor("zt2", [P, R], FP8, kind="ExternalInput")
    # interleaved per-tile masks: mab[p, CK*t+j] = (batch[t*128+p] == A_t + j)
    mab1 = nc.dram_tensor("mab1", [P, NT * CK], FP8, kind="ExternalInput")
    mab2 = nc.dram_tensor("mab2", [P, NT * CK], FP8, kind="ExternalInput")
    mabb1 = nc.dram_tensor("mabb1", [P, NT * CK], BF16, kind="ExternalInput")
    mabb2 = nc.dram_tensor("mabb2", [P, NT * CK], BF16, kind="ExternalInput")
    # scatter one-hot: sel[c, w] = 1 iff a_{c//CK} + c%CK == w (local window)
    sel1 = nc.dram_tensor("sel1", [2 * P, GPC], F32, kind="ExternalInput")
    sel2 = nc.dram_tensor("sel2", [2 * P, GPC], F32, kind="ExternalInput")
    # gather one-hot: e_s[w, j] = 1 iff candidate col j sources side s graph w
    e1 = nc.dram_tensor("e1", [GPC, NIDXP], BF16, kind="ExternalInput")
    e2 = nc.dram_tensor("e2", [GPC, NIDXP], BF16, kind="ExternalInput")
    out_part = nc.dram_tensor("out_part", [2, 1], F32, kind="ExternalOutput")

    zs = [z1, z2]
    zts = [zt1, zt2]
    mabs = [mab1, mab2]
    mabbs = [mabb1, mabb2]
    sels = [sel1, sel2]
    es = [e1, e2]

    def _body(tc):
        with (
            tc.tile_pool(name="const", bufs=1) as constp,
            tc.tile_pool(name="stash", bufs=1) as stashp,
            tc.tile_pool(name="zin", bufs=3) as zinp,
            tc.tile_pool(name="sq", bufs=2) as sqp,
            tc.tile_pool(name="small", bufs=4) as smallp,
            tc.tile_pool(name="pstag", bufs=1, space="PSUM") as pstagp,
            tc.tile_pool(name="pcand", bufs=1, space="PSUM") as pcandp,
            tc.tile_pool(name="pg", bufs=1, space="PSUM") as pgp,
            tc.tile_pool(name="pssq", bufs=1, space="PSUM") as pssqp,
            tc.tile_pool(name="pzt", bufs=2, space="PSUM") as pztp,
        ):
            # ---- constants / small inputs to SBUF (sync queue) ----
            ident = constp.tile([P, P], F32)
            make_identity(nc, ident[:])
            ones_col = constp.tile([P, 1], F32)
            nc.vector.memset(ones_col[:], 1.0)
            ones_bf = constp.tile([P, 1], BF16)
            nc.vector.memset(ones_bf[:], 1.0)

            mab_sb = []
            mabb_sb = []
            for s in range(2):
                m = constp.tile([P, NT * CK], FP8, name=f"mab_sb{s}")
                nc.sync.dma_start(out=m[:], in_=mabs[s][:])
                mab_sb.append(m)
                mb = constp.tile([P, NT * CK], BF16, name=f"mabb_sb{s}")
                nc.sync.dma_start(out=mb[:], in_=mabbs[s][:])
                mabb_sb.append(mb)

            # ---- persistent stashes ----
            zst = [stashp.tile([P, R], FP8, name=f"zst{s}") for s in range(2)]
            ssq = stashp.tile([P, 2 * NT], F32, name="ssq")
            cand8 = stashp.tile([P, NT * 8], F32, name="cand8")
            gselb = stashp.tile([P, NIDXP], FP8, name="gselb")

            # ---- DMA dispatch: z natural chunks (sync), zT (scalar queue) ----
            znat_bufs = []   # (t0, w, [tile per side])
            t0 = 0
            for gi, w in enumerate(CHN):
                row = []
                for s in range(2):
                    zg = zinp.tile([P, 18 * P], FP8, tag=f"zn{s}")
                    nc.sync.dma_start(
                        out=zg[:, : w * P], in_=zs[s][:, t0 * P : (t0 + w) * P]
                    )
                    row.append(zg)
                znat_bufs.append((t0, w, row))
                t0 += w
            t0 = 0
            for gi, w in enumerate(CHT):
                for s in range(2):
                    nc.scalar.dma_start(
                        out=zst[s][:, t0 * P : (t0 + w) * P],
                        in_=zts[s][:, t0 * P : (t0 + w) * P],
                    )
                t0 += w
            sel_sb = []
            e_sb = []
            for s in range(2):
                s0 = constp.tile([P, GPC], F32, name=f"sel_sb{s}a")
                s1 = constp.tile([P, GPC], F32, name=f"sel_sb{s}b")
                nc.sync.dma_start(out=s0[:], in_=sels[s][0:P, :])
                nc.sync.dma_start(out=s1[:], in_=sels[s][P : 2 * P, :])
                sel_sb.append((s0, s1))
                e = constp.tile([GPC, NIDXP], BF16, name=f"e_sb{s}")
                nc.sync.dma_start(out=e[:], in_=es[s][:])
                e_sb.append(e)

            # ---- P1 + S: seg matmuls, squares, row-sum matmuls ----
            pstag = [
                pstagp.tile([P, NT * CK], F32, name=f"pstag{s}") for s in range(2)
            ]
            pssq = pssqp.tile([P, 2 * NT], F32, name="pssq")
            for t0, w, row in znat_bufs:
                for s in range(2):
                    for k in range(w):
                        t = t0 + k
                        nc.tensor.matmul(
                            out=pstag[s][:, CK * t : CK * (t + 1)],
                            lhsT=row[s][:, k * P : (k + 1) * P],
                            rhs=mab_sb[s][:, CK * t : CK * (t + 1)],
                            start=True,
                            stop=True,
                        )
                # squares: side 0 on Scalar, side 1 on DVE
                sq0 = sqp.tile([P, 18 * P], BF16, tag="sq0")
                nc.scalar.activation(
                    out=sq0[:, : w * P], in_=row[0][:, : w * P], func=AF.Square
                )
                sq1 = sqp.tile([P, 18 * P], BF16, tag="sq1")
                nc.vector.tensor_tensor(
                    out=sq1[:, : w * P], in0=row[1][:, : w * P],
                    in1=row[1][:, : w * P], op=AL.mult,
                )
                for s, sq in ((0, sq0), (1, sq1)):
                    for k in range(w):
                        t = t0 + k
                        nc.tensor.matmul(
                            out=pssq[:, s * NT + t : s * NT + t + 1],
                            lhsT=sq[:, k * P : (k + 1) * P],
                            rhs=ones_bf[:],
                            start=True,
                            stop=True,
                        )

            # ---- P2: transpose stag, scatter into local g ----
            pg = pgp.tile([GPC, 2 * P], F32, name="pg")
            for s in range(2):
                stag_sb = smallp.tile([P, NT * CK], F32, tag="stag", bufs=2)
                nc.scalar.copy(out=stag_sb[:], in_=pstag[s][:])
                rem = NT * CK - P  # 78
                stg_a = smallp.tile([P, P], F32, tag="stg", bufs=2)
                stg_b = smallp.tile([P, P], F32, tag="stg", bufs=2)
                pta = pztp.tile([P, P], F32, tag="pzt")
                nc.tensor.transpose(out=pta[:], in_=stag_sb[:, 0:P], identity=ident[:])
                nc.scalar.copy(out=stg_a[:], in_=pta[:])
                ptb = pztp.tile([P, P], F32, tag="pzt")
                nc.tensor.transpose(
                    out=ptb[:rem, :], in_=stag_sb[:, P : NT * CK], identity=ident[:]
                )
                nc.scalar.copy(out=stg_b[:rem, :], in_=ptb[:rem, :])
                nc.tensor.matmul(
                    out=pg[:, s * P : (s + 1) * P],
                    lhsT=sel_sb[s][0][:],
                    rhs=stg_a[:],
                    start=True,
                    stop=False,
                )
                nc.tensor.matmul(
                    out=pg[:, s * P : (s + 1) * P],
                    lhsT=sel_sb[s][1][:rem, :],
                    rhs=stg_b[:rem, :],
                    start=False,
                    stop=True,
                )

            # ---- P3: normalize g rows, cast to bf16 ----
            gf = smallp.tile([GPC, 2 * P], F32, tag="gf")
            nc.scalar.copy(out=gf[:], in_=pg[:])
            gnb = smallp.tile([GPC, 2 * P], BF16, tag="gnb")
            gsq = smallp.tile([GPC, 2 * P], F32, tag="gsq")
            gss = smallp.tile([GPC, 2], F32, tag="gss")
            for s in range(2):
                nc.scalar.activation(
                    out=gsq[:, s * P : (s + 1) * P],
                    in_=gf[:, s * P : (s + 1) * P],
                    func=AF.Square, accum_out=gss[:, s : s + 1],
                )
            gnr = smallp.tile([GPC, 2], F32, tag="gnrm", bufs=4)
            nc.scalar.sqrt(out=gnr[:], in_=gss[:])
            # row norms of z (ssq ready once P1 chunks done); same Sqrt table
            ssq_sb = smallp.tile([P, 2 * NT], F32, tag="rn", bufs=4)
            nc.vector.tensor_copy(out=ssq_sb[:], in_=pssq[:])
            rn = smallp.tile([P, 2 * NT], F32, tag="rn", bufs=4)
            nc.scalar.sqrt(out=rn[:], in_=ssq_sb[:])
            nc.vector.tensor_scalar(
                out=gnr[:], in0=gnr[:], scalar1=EPS, scalar2=None, op0=AL.max
            )
            ginv = smallp.tile([GPC, 2], F32, tag="gnrm", bufs=4)
            nc.vector.reciprocal(out=ginv[:], in_=gnr[:])
            for s in range(2):
                nc.scalar.mul(
                    out=gnb[:, s * P : (s + 1) * P],
                    in_=gf[:, s * P : (s + 1) * P],
                    mul=ginv[:, s : s + 1],
                )
            nc.vector.tensor_scalar(
                out=rn[:], in0=rn[:], scalar1=EPS, scalar2=None, op0=AL.max
            )
            inv = smallp.tile([P, 2 * NT], F32, tag="rn", bufs=4)
            nc.vector.reciprocal(out=inv[:], in_=rn[:])

            # ---- P5: gather candidate columns via one-hot matmuls ----
            H = NIDXP // 2  # 416
            for h in range(2):
                pgs = pztp.tile([P, H], F32, tag="pzt")
                for s in range(2):
                    nc.tensor.matmul(
                        out=pgs[:],
                        lhsT=gnb[:, s * P : (s + 1) * P],
                        rhs=e_sb[s][:, h * H : (h + 1) * H],
                        start=(s == 0),
                        stop=(s == 1),
                    )
                if h == 0:
                    nc.scalar.copy(out=gselb[:, h * H : (h + 1) * H], in_=pgs[:])
                else:
                    nc.vector.tensor_copy(
                        out=gselb[:, h * H : (h + 1) * H], in_=pgs[:]
                    )

            # ---- P6: per-tile matvecs, region-wise into 2 PSUM banks ----
            pcand = [
                pcandp.tile([P, HT * 8], F32, name=f"pcand{h}") for h in range(2)
            ]
            for t in range(NT):
                h, o = t // HT, (t % HT) * 8
                for s in range(2):
                    nc.tensor.matmul(
                        out=pcand[h][:, o + 4 * s : o + 4 * s + 4],
                        lhsT=zst[s][:, t * P : (t + 1) * P],
                        rhs=gselb[:, (t * 2 + s) * 4 : (t * 2 + s) * 4 + 4],
                        start=True,
                        stop=True,
                    )
            nc.vector.tensor_copy(out=cand8[:, : HT * 8], in_=pcand[0][:])
            nc.vector.tensor_copy(
                out=cand8[:, HT * 8 : NT * 8], in_=pcand[1][:, : (NT - HT) * 8]
            )

            # ---- P7: select (mask folded with 1/||z||), softplus, reduce ----
            # minv[p, 2t+j] = mab[p, 2t+j] * inv[p, t]
            inv2 = smallp.tile([P, NT * CK], F32, tag="inv2", bufs=2)
            minv = smallp.tile([P, NT * CK], F32, tag="minv", bufs=2)
            psel = smallp.tile([P, 2 * NT * CK], F32, tag="psel")
            spv = smallp.tile([P, 2 * NT * CK], F32, tag="spv")
            i2v = inv2[:].rearrange("p (t j) -> p t j", j=CK)
            cv = cand8[:].rearrange("p (t w) -> p t w", w=8)
            pv = psel[:].rearrange("p (s t j) -> p s t j", s=2, j=CK)
            for s in range(2):
                iv = inv[:, s * NT : (s + 1) * NT]
                for j in range(CK):
                    nc.vector.tensor_copy(out=i2v[:, :, j], in_=iv)
                nc.vector.tensor_tensor(
                    out=minv[:], in0=mabb_sb[s][:], in1=inv2[:], op=AL.mult
                )
                mv = minv[:].rearrange("p (t j) -> p t j", j=CK)
                ta4 = smallp.tile([P, NT * 4], F32, tag="ta4", bufs=2)
                t4 = ta4[:].rearrange("p (t q) -> p t q", q=4)
                nc.vector.tensor_tensor(
                    out=t4[:, :, 0:2], in0=mv[:], in1=cv[:, :, 4 * s : 4 * s + 2],
                    op=AL.mult,
                )
                nc.vector.tensor_tensor(
                    out=t4[:, :, 2:4], in0=mv[:], in1=cv[:, :, 4 * s + 2 : 4 * s + 4],
                    op=AL.mult,
                )
                # pairsum -> (pos, cross) interleaved
                nc.vector.tensor_tensor(
                    out=pv[:, s], in0=t4[:, :, 0::2], in1=t4[:, :, 1::2], op=AL.add
                )
            # q = ln(sigmoid(x)) = -softplus(-x), batched over both sides
            nc.scalar.activation(out=spv[:], in_=psel[:], func=AF.Sigmoid)
            nc.scalar.activation(out=spv[:], in_=spv[:], func=AF.Ln)
            sv = spv[:].rearrange("p (s t j) -> p s t j", s=2, j=CK)
            d2col = smallp.tile([P, 2], F32, tag="d2col")
            for s in range(2):
                # d = sp(-cross) - sp(-pos) = q_pos - q_cross
                dd = smallp.tile([P, NT], F32, tag="fin", bufs=4)
                nc.vector.tensor_tensor(
                    out=dd[:], in0=sv[:, s, :, 0], in1=sv[:, s, :, 1], op=AL.subtract
                )
                dsq = smallp.tile([P, NT], F32, tag="dsq", bufs=2)
                nc.scalar.activation(
                    out=dsq[:], in_=dd[:], func=AF.Square,
                    accum_out=d2col[:, s : s + 1],
                )

            pfin = pztp.tile([2, 1], F32, tag="pzt")
            nc.tensor.matmul(
                out=pfin[:], lhsT=d2col[:], rhs=ones_col[:], start=True, stop=True
            )
            osb = smallp.tile([2, 1], F32, tag="osb")
            nc.vector.tensor_copy(out=osb[:], in_=pfin[:])
            nc.sync.dma_start(out=out_part[:], in_=osb[:])

    with tile.TileContext(nc) as tc:
        _body(tc)
    if finalize:
        nc.finalize()
    return nc


def prep_inputs(z1, z2, batch_1, batch_2):
    """Graph-aligned shards + all index-derived input tensors (host-side)."""
    z1 = np.asarray(z1, dtype=np.float32)
    z2 = np.asarray(z2, dtype=np.float32)
    b1 = np.asarray(batch_1).astype(np.int64)
    b2 = np.asarray(batch_2).astype(np.int64)
    FP8H = ml_dtypes.float8_e4m3fn

    in_maps = []
    for c in range(NCORES):
        glo, ghi = c * GPC, (c + 1) * GPC
        m = {}
        idx_cols = np.zeros((NT, 2, 2 * CK), dtype=np.int64)
        for s, (z, b) in enumerate(((z1, b1), (z2, b2))):
            lo, hi = np.searchsorted(b, [glo, ghi])
            cnt = hi - lo
            assert cnt <= R, f"core {c} side {s}: {cnt} rows > {R}"
            zp = np.zeros((R, D), dtype=FP8H)
            zp[:cnt] = z[lo:hi].astype(FP8H)
            # tile-interleaved natural layout [128, R]
            m[f"z{s + 1}"] = np.ascontiguousarray(
                zp.reshape(NT, P, D).transpose(1, 0, 2).reshape(P, R)
            )
            m[f"zt{s + 1}"] = np.ascontiguousarray(zp.T)
            bt = np.full((R,), -1, dtype=np.int64)
            bt[:cnt] = b[lo:hi]
            btt = bt.reshape(NT, P)
            A = btt[:, 0].copy()
            A[A < 0] = glo
            vmax = btt.max(axis=1)
            assert (vmax - A <= CK - 1).all(), "tile spans >CK graphs"
            a = A - glo
            assert (a >= 0).all() and (a < GPC).all()
            mab = np.zeros((P, NT * CK), dtype=np.float32)
            for j in range(CK):
                mab[:, j::CK] = (btt == (A + j)[:, None]).T.astype(np.float32)
            m[f"mab{s + 1}"] = mab.astype(FP8H)
            m[f"mabb{s + 1}"] = mab.astype(ml_dtypes.bfloat16)
            sel = np.zeros((2 * P, GPC), dtype=np.float32)
            crow = np.arange(NT * CK)
            gid = np.repeat(a, CK) + np.tile(np.arange(CK), NT)
            ok = gid < GPC
            sel[crow[ok], gid[ok]] = 1.0
            m[f"sel{s + 1}"] = sel
            # candidate cols j = t*8 + side*4 + q: [own A, own A+1, other A, other A+1]
            idx_cols[:, s, :CK] = a[:, None] + np.arange(CK)
            idx_cols[:, s, CK:] = a[:, None] + np.arange(CK)
        # gather one-hots: E_s[w, j] = 1 iff col j sources side s, graph w
        for s in range(2):
            E = np.zeros((GPC, NIDXP), dtype=ml_dtypes.bfloat16)
            for t in range(NT):
                for side in range(2):
                    for q in range(2 * CK):
                        src = side if q < CK else 1 - side
                        if src != s:
                            continue
                        w = idx_cols[t, side, q]
                        if w < GPC:
                            E[w, t * 8 + side * 4 + q] = 1.0
            m[f"e{s + 1}"] = E
        in_maps.append(m)
    return in_maps


_NC_CACHE = {}


def _get_nc():
    if "nc" not in _NC_CACHE:
        _NC_CACHE["nc"] = build_nc()
    return _NC_CACHE["nc"]


def kernel(z1, z2, batch_1, batch_2):
    nc = _get_nc()
    in_maps = prep_inputs(z1, z2, batch_1, batch_2)
    res = run_bass_kernel_spmd(nc, in_maps, list(range(NCORES)))
    parts = np.stack([r["out_part"].reshape(2) for r in res.results])  # [8, 2]
    tot = parts.sum(axis=0)
    return np.float32(np.sqrt(tot[0]) + np.sqrt(tot[1]))
